# revision 13
# baseline (speedup 1.0000x reference)
"""GroupQueryAttention TRN2 Bass kernel (optimized; baseline v1 was ~329us).

Problem: B=4, T=2048, C=1024, H=16 heads, G=4 groups, head_dim=64, causal.
Sharding: 8 cores = 4 batches (DP) x 2 tensor-parallel halves (8 heads /
2 groups per core). Host converts inputs to bf16 and pre-transposes; each
core computes a partial output projection over its 512 attention channels;
host upcasts, sums the two TP partials per batch, and adds the bias.

Design:
- all-bf16 datapath: halves DMA bytes, 1 col/cycle matmuls (fp32r measured
  ~1.5 cyc/col on HW), bf16 y output summed in f32 on host.
- head pairs (2p4, 2p4+1) share one [128, 1024] psum score tile (2 banks):
  off-diagonal steps get ONE 1024-col exp ACTIVATE for both heads, halving
  the ACT engine's ~352-cycle/instruction overhead (ACT was v1's pacer).
- skew-1 software pipelining: scores for step t are emitted before PV of
  step t-1, so the strict-FIFO PE never head-of-line blocks on the exp.
- score matmul pairs sit on row bands 0:64/64:128 (kdup duplication) and are
  emitted back-to-back: the PE overlaps them (~3ns stagger, row_grp tiling).
- softmax denominators ride free in the PV matmuls: v_sb lhsT columns are
  [ones64 | v64], so psum rows 0:64 accumulate the denominator at zero extra
  PE cycles; reciprocal_approx_fast (kept at base partition 0 - cross-base
  recip is silently broken on HW) + tensor_mul normalize.
- 12 warmup matmuls on a memset tile open the kernel so the HAM clock gate
  releases (~3.4us sustained activity -> 2.4 GHz) while input DMAs stream.
- phase fusion via generator feeds: projections for later tq blocks and
  output projections for earlier blocks are interleaved into the attention
  steps at tuned rates, keeping PE and ACT simultaneously busy end-to-end.
"""

import sys
import numpy as np
import ml_dtypes

for _p in ("/opt/trn_rl_repo", "/opt/trn_rl_repo/concourse"):
    if _p not in sys.path:
        sys.path.insert(0, _p)

import concourse.bass as bass  # noqa: E402
import concourse.mybir as mybir  # noqa: E402
from concourse import bacc  # noqa: E402
from concourse.tile import TileContext  # noqa: E402
from concourse.bass_utils import run_bass_kernel_spmd  # noqa: E402
from concourse.masks import make_identity, make_upper_triangular  # noqa: E402

F32 = mybir.dt.float32
BF16 = mybir.dt.bfloat16
EXP = mybir.ActivationFunctionType.Exp

B, T, C = 4, 2048, 1024
NH, NG, HD = 16, 4, 64
NH_LOC, NG_LOC = 8, 2          # per-core heads / groups
S = NH_LOC * HD                # 512 local attention channels
TQB = 512                      # tq block
NTQB = T // TQB                # 4
NCT = C // 128                 # 8 contraction tiles
SCALE = float(HD) ** -0.5
N_WARMUP = 12


def _build_program():
    nc = bacc.Bacc("TRN2", target_bir_lowering=False, debug=False, num_devices=8)

    xT = nc.dram_tensor("xT", [C, T], BF16, kind="ExternalInput")
    wqT = nc.dram_tensor("wqT", [C, S], BF16, kind="ExternalInput")
    wkT = nc.dram_tensor("wkT", [C, NG_LOC * HD], BF16, kind="ExternalInput")
    wvT = nc.dram_tensor("wvT", [C, NG_LOC * HD], BF16, kind="ExternalInput")
    wpT = nc.dram_tensor("wpT", [S, C], BF16, kind="ExternalInput")
    y = nc.dram_tensor("y", [T, C], BF16, kind="ExternalOutput")

    with TileContext(nc) as tc:
        with tc.tile_pool(name="const", bufs=1) as const_pool, \
             tc.tile_pool(name="persist", bufs=1) as persist, \
             tc.tile_pool(name="vtp", bufs=2) as vtp, \
             tc.tile_pool(name="pp", bufs=3) as ppool, \
             tc.tile_pool(name="attn", bufs=4) as apool, \
             tc.tile_pool(name="sm", bufs=2) as small, \
             tc.tile_pool(name="yo", bufs=3) as ypool, \
             tc.tile_pool(name="psS", bufs=2, space="PSUM") as psS, \
             tc.tile_pool(name="psO", bufs=1, space="PSUM") as psO, \
             tc.tile_pool(name="psM", bufs=1, space="PSUM") as psM, \
             tc.tile_pool(name="psT", bufs=1, space="PSUM") as psT:

            # ---- warmup first: PE busy from t~0 releases the HAM gate ----
            wtile = const_pool.tile([128, 512], BF16)
            nc.vector.memset(wtile, 0.125)
            for _ in range(N_WARMUP):
                pswu = psM.tile([128, 512], F32, tag="mm", name="pswu")
                nc.tensor.matmul(pswu, wtile[:, 0:128], wtile,
                                 start=True, stop=True)

            # ---- constants ----
            ident = const_pool.tile([128, 64], F32)
            make_identity(nc, ident[0:64, 0:64])
            make_identity(nc, ident[64:128, 0:64], nomemset=False)
            mask32 = const_pool.tile([128, 128], F32)
            make_upper_triangular(nc, mask32, val=1.0, diag=True)
            mask = const_pool.tile([128, 128], BF16)
            nc.vector.tensor_copy(mask, mask32)
            # ---- persistent SBUF ----
            qt = [persist.tile([128, T], BF16, tag=f"qt{i}", name=f"qt{i}")
                  for i in range(4)]
            kdup = [persist.tile([128, T], BF16, tag=f"kd{g}", name=f"kd{g}")
                    for g in range(NG_LOC)]
            v_sb = [persist.tile([128, T], BF16, tag=f"v{g}", name=f"v{g}")
                    for g in range(NG_LOC)]
            xts = [persist.tile([128, T], BF16, tag=f"x{ct}", name=f"x{ct}")
                   for ct in range(NCT)]
            wq_sb = [persist.tile([128, S], BF16, tag=f"wq{ct}", name=f"wq{ct}")
                     for ct in range(NCT)]
            wk_sb = [persist.tile([128, 128], BF16, tag=f"wk{ct}", name=f"wk{ct}")
                     for ct in range(NCT)]
            wv_sb = [persist.tile([128, 128], BF16, tag=f"wv{ct}", name=f"wv{ct}")
                     for ct in range(NCT)]
            wp_sb = [persist.tile([128, C], BF16, tag=f"wp{i}", name=f"wp{i}")
                     for i in range(4)]

            # ---- DMAs: block-0 x first (unblocks A0), weights, rest of x ----
            for ct in range(NCT):
                nc.sync.dma_start(out=xts[ct][:, 0:512],
                                  in_=xT[ct * 128:(ct + 1) * 128, 0:512])
            for ct in range(NCT):
                nc.sync.dma_start(out=wq_sb[ct], in_=wqT[ct * 128:(ct + 1) * 128, :])
                nc.sync.dma_start(out=wk_sb[ct], in_=wkT[ct * 128:(ct + 1) * 128, :])
                nc.sync.dma_start(out=wv_sb[ct], in_=wvT[ct * 128:(ct + 1) * 128, :])
            for ct in range(NCT):
                nc.sync.dma_start(out=xts[ct][:, 512:1024],
                                  in_=xT[ct * 128:(ct + 1) * 128, 512:1024])
            for ct in range(NCT):
                nc.sync.dma_start(out=xts[ct][:, 1024:2048],
                                  in_=xT[ct * 128:(ct + 1) * 128, 1024:2048])
            for i in range(4):
                nc.sync.dma_start(out=wp_sb[i], in_=wpT[i * 128:(i + 1) * 128, :])

            # ones columns of v_sb (denominator trick)
            ones64 = const_pool.tile([128, 64], F32)
            nc.vector.memset(ones64, 1.0)
            for g in range(NG_LOC):
                for t in range(T // 128):
                    nc.vector.tensor_copy(
                        v_sb[g][:, t * 128:t * 128 + 64], ones64)

            # extra warmups threaded into A0: bridge DMA stalls so the
            # HAM activity window never sees an idle gap while A0 streams in
            warm_budget = [14]

            def emit_warm():
                if warm_budget[0] > 0:
                    warm_budget[0] -= 1
                    pswu = psM.tile([128, 512], F32, tag="mm", name="pswu")
                    nc.tensor.matmul(pswu, wtile[:, 0:128], wtile,
                                     start=True, stop=True)

            # ---- generators for interleavable PE work ----
            def proj_block(j, use_s_pool):
                """Projections q/k/v for tq/tk block j + v transpose."""
                cols = slice(j * TQB, (j + 1) * TQB)

                def fresh():
                    if use_s_pool:
                        psx = psS.tile([128, 2 * TQB], F32, tag="s", name="psx")
                        return psx[:, 0:TQB]
                    return psM.tile([128, TQB], F32, tag="mm", name="psm")

                for p4 in range(4):
                    dst = fresh()
                    for ct in range(NCT):
                        nc.tensor.matmul(
                            dst, wq_sb[ct][:, p4 * 128:(p4 + 1) * 128],
                            xts[ct][:, cols], start=(ct == 0), stop=(ct == NCT - 1))
                        if use_s_pool and ct % 2 == 1:
                            emit_warm()
                        yield
                    nc.vector.tensor_copy(qt[p4][:, cols], dst)
                # k (both groups in one psum: g0 on 0:64, g1 on 64:128)
                dst = fresh()
                for ct in range(NCT):
                    nc.tensor.matmul(dst, wk_sb[ct], xts[ct][:, cols],
                                     start=(ct == 0), stop=(ct == NCT - 1))
                    yield
                for g in range(NG_LOC):
                    rows = slice(g * 64, (g + 1) * 64)
                    nc.vector.tensor_copy(kdup[g][0:64, cols], dst[rows, :])
                    nc.vector.tensor_copy(kdup[g][64:128, cols], dst[rows, :])
                # v -> vt (sbuf) -> per-128-block transpose into v_sb
                dst = fresh()
                for ct in range(NCT):
                    nc.tensor.matmul(dst, wv_sb[ct], xts[ct][:, cols],
                                     start=(ct == 0), stop=(ct == NCT - 1))
                    yield
                vt = vtp.tile([128, TQB], F32, tag="vt", name="vt")
                nc.vector.tensor_copy(vt, dst)
                for g in range(NG_LOC):
                    for ts_ in range(4):
                        t_abs = 4 * j + ts_
                        pst = psT.tile([128, 512], F32, tag="tr", name="pst")
                        nc.tensor.transpose(
                            pst[:, 0:64],
                            vt[g * 64:(g + 1) * 64, ts_ * 128:(ts_ + 1) * 128],
                            ident[g * 64:(g + 1) * 64, 0:64])
                        yield
                        nc.vector.tensor_copy(
                            v_sb[g][:, t_abs * 128 + 64:(t_abs + 1) * 128],
                            pst[:, 0:64])

            def outproj_block(j, at_tiles):
                """Output projection for tq block j (4 tau rows of 128)."""
                for tt in range(4):
                    tau = 4 * j + tt
                    ysb = ypool.tile([128, C], BF16, tag="y", name="ysb")
                    for half in range(2):
                        if (tt * 2 + half) % 2 == 0:
                            yp = psM.tile([128, TQB], F32, tag="mm", name="yp")
                        else:
                            yp = psT.tile([128, TQB], F32, tag="tr", name="yp")
                        for p4 in range(4):
                            nc.tensor.matmul(
                                yp, at_tiles[p4][:, tt * 128:(tt + 1) * 128],
                                wp_sb[p4][:, half * TQB:(half + 1) * TQB],
                                start=(p4 == 0), stop=(p4 == 3))
                            yield
                        cols = slice(half * TQB, (half + 1) * TQB)
                        nc.vector.tensor_copy(ysb[:, cols], yp)
                        nc.sync.dma_start(
                            out=y[tau * 128:(tau + 1) * 128, cols],
                            in_=ysb[:, cols])

            # ---- attention ----
            def emit_e(j, t, ps):
                """exp + mask for step t; returns the pt tile."""
                c = t - 4 * j
                off = max(0, c * 128)
                pt = ppool.tile([128, 2 * TQB], BF16, tag="pt", name="pt")
                if off == 0:
                    # both heads' regions are contiguous: one 1024-col exp
                    nc.scalar.activation(pt[:, :], ps[:, :], EXP, scale=SCALE)
                else:
                    nc.scalar.activation(pt[:, off:TQB], ps[:, off:TQB],
                                         EXP, scale=SCALE)
                    nc.scalar.activation(pt[:, TQB + off:2 * TQB],
                                         ps[:, TQB + off:2 * TQB],
                                         EXP, scale=SCALE)
                if c >= 0:
                    nc.vector.tensor_mul(
                        pt[:, off:off + 128], pt[:, off:off + 128], mask)
                    nc.gpsimd.tensor_mul(
                        pt[:, TQB + off:TQB + off + 128],
                        pt[:, TQB + off:TQB + off + 128], mask)
                return pt

            def emit_p(j, p4, t, pt, po, ntk, rcp_tile=None):
                """PV pair for step t of pair p4, block j."""
                g = p4 // 2
                c = t - 4 * j
                off = max(0, c * 128)
                for h01 in range(2):
                    nc.tensor.matmul(
                        po[:, h01 * TQB + off:(h01 + 1) * TQB],
                        v_sb[g][:, t * 128:(t + 1) * 128],
                        pt[:, h01 * TQB + off:(h01 + 1) * TQB],
                        start=(t == 0), stop=(t == ntk - 1))
                    if t == ntk - 1 and rcp_tile is not None:
                        nc.vector.reciprocal_approx_fast(
                            rcp_tile[0:64, h01 * TQB:(h01 + 1) * TQB],
                            po[0:64, h01 * TQB:(h01 + 1) * TQB])

            y3acc = [persist.tile([128, C], F32, tag=f"y3a{tt}",
                                  name=f"y3a{tt}") for tt in range(4)]

            def outproj3_pair(p4, at_tiles):
                """One pair's contribution to block-3 output projection,
                accumulated in SBUF so it interleaves into B3's steps."""
                for tt in range(4):
                    for half in range(2):
                        cols = slice(half * TQB, (half + 1) * TQB)
                        if (tt * 2 + half) % 2 == 0:
                            yp = psM.tile([128, TQB], F32, tag="mm", name="yp")
                        else:
                            yp = psT.tile([128, TQB], F32, tag="tr", name="yp")
                        nc.tensor.matmul(
                            yp, at_tiles[p4][:, tt * 128:(tt + 1) * 128],
                            wp_sb[p4][:, cols], start=True, stop=True)
                        yield
                        if p4 == 0:
                            nc.vector.tensor_copy(y3acc[tt][:, cols], yp)
                        else:
                            nc.vector.tensor_add(
                                y3acc[tt][:, cols], y3acc[tt][:, cols], yp)
                    if p4 == 3:
                        tau = 12 + tt
                        ysb = ypool.tile([128, C], BF16, tag="y", name="ysb")
                        nc.vector.tensor_copy(ysb, y3acc[tt])
                        nc.sync.dma_start(
                            out=y[tau * 128:(tau + 1) * 128, :], in_=ysb)

            def attention_block(j, feed, rate):
                """Attention for tq block j; drains `feed` generators at
                ~`rate` PE ops per step."""
                tq0 = j * TQB
                ntk = 4 * (j + 1)
                at_tiles = [apool.tile([128, TQB], BF16, tag=f"at{p4}",
                                       name=f"at{j}_{p4}")
                            for p4 in range(4)]
                budget = 0.0
                for p4 in range(4):
                    g = p4 // 2
                    po = psO.tile([128, 2 * TQB], F32, tag="po", name="po")
                    ps_by_t = {}
                    pt_by_t = {}
                    for t in range(ntk):
                        c = t - 4 * j
                        off = max(0, c * 128)
                        ps = psS.tile([128, 2 * TQB], F32, tag="s", name="ps")
                        nc.tensor.matmul(
                            ps[:, off:TQB],
                            kdup[g][0:64, t * 128:(t + 1) * 128],
                            qt[p4][0:64, tq0 + off:tq0 + TQB],
                            start=True, stop=True)
                        nc.tensor.matmul(
                            ps[:, TQB + off:2 * TQB],
                            kdup[g][64:128, t * 128:(t + 1) * 128],
                            qt[p4][64:128, tq0 + off:tq0 + TQB],
                            start=True, stop=True)
                        ps_by_t[t] = ps
                        if t >= 1:
                            pt_by_t[t - 1] = emit_e(j, t - 1, ps_by_t.pop(t - 1))
                        if t >= 2:
                            emit_p(j, p4, t - 2, pt_by_t.pop(t - 2), po, ntk)
                        budget += rate
                        while budget >= 1.0 and feed:
                            try:
                                next(feed[0])
                                budget -= 1.0
                            except StopIteration:
                                feed.pop(0)
                    rcp = small.tile([128, 2 * TQB], F32, tag="rcp", name="rcp")
                    pt_by_t[ntk - 1] = emit_e(j, ntk - 1, ps_by_t.pop(ntk - 1))
                    emit_p(j, p4, ntk - 2, pt_by_t.pop(ntk - 2), po, ntk)
                    emit_p(j, p4, ntk - 1, pt_by_t.pop(ntk - 1), po, ntk,
                           rcp_tile=rcp)
                    # normalization (recips were emitted inside emit_p)
                    nc.vector.tensor_mul(
                        at_tiles[p4][0:64, :], po[64:128, 0:TQB],
                        rcp[0:64, 0:TQB])
                    nc.vector.tensor_mul(
                        at_tiles[p4][64:128, :], po[64:128, TQB:2 * TQB],
                        rcp[0:64, TQB:2 * TQB])
                    if j == 3:
                        feed.append(outproj3_pair(p4, at_tiles))
                return at_tiles

            def drain(gen):
                for _ in gen:
                    pass

            # ---- schedule ----
            drain(proj_block(0, use_s_pool=True))
            feed = [proj_block(1, use_s_pool=False),
                    proj_block(2, use_s_pool=False),
                    proj_block(3, use_s_pool=False)]
            at0 = attention_block(0, feed, 3.8)
            feed.append(outproj_block(0, at0))
            at1 = attention_block(1, feed, 1.45)
            feed.append(outproj_block(1, at1))
            at2 = attention_block(2, feed, 1.3)
            feed.append(outproj_block(2, at2))
            at3 = attention_block(3, feed, 1.3)
            for gen in feed:
                drain(gen)

    nc.compile()
    return nc


_NC_CACHE = None


def _get_nc():
    global _NC_CACHE
    if _NC_CACHE is None:
        _NC_CACHE = _build_program()
    return _NC_CACHE


def _bf16(a):
    return np.ascontiguousarray(a).astype(ml_dtypes.bfloat16)


def _make_in_maps(x, Wq, Wk, Wv, Wp):
    in_maps = []
    for core in range(8):
        b, tp = core // 2, core % 2
        hs = slice(tp * NH_LOC, (tp + 1) * NH_LOC)
        gs = slice(tp * NG_LOC, (tp + 1) * NG_LOC)
        in_maps.append({
            "xT": _bf16(x[b].T),
            "wqT": _bf16(Wq[hs].transpose(2, 0, 1).reshape(C, S)),
            "wkT": _bf16(Wk[gs].transpose(2, 0, 1).reshape(C, NG_LOC * HD)),
            "wvT": _bf16(Wv[gs].transpose(2, 0, 1).reshape(C, NG_LOC * HD)),
            "wpT": _bf16(Wp[:, tp * S:(tp + 1) * S].T),
        })
    return in_maps


def kernel(x, Wq, Wk, Wv, Wp, bp, _trace=False):
    x = np.asarray(x, dtype=np.float32)
    nc = _get_nc()
    in_maps = _make_in_maps(
        x, np.asarray(Wq, np.float32), np.asarray(Wk, np.float32),
        np.asarray(Wv, np.float32), np.asarray(Wp, np.float32))
    res = run_bass_kernel_spmd(nc, in_maps, list(range(8)), trace=_trace)
    out = np.empty((B, T, C), dtype=np.float32)
    bp32 = np.asarray(bp, np.float32)
    for b in range(B):
        out[b] = (res.results[2 * b]["y"].astype(np.float32)
                  + res.results[2 * b + 1]["y"].astype(np.float32) + bp32)
    if _trace:
        return out, res
    return out
